# revision 37
# baseline (speedup 1.0000x reference)
"""Trainium2 Bass kernel for the periodic flux-divergence stencil:

    out = sum_ax  (v - roll(v, 1, ax)),  v = 0.5*(roll(M,-1,ax)+M)*(roll(mu,-1,ax)-mu)

over axes H, W of [B=16, 1, 1024, 1024] f32 inputs, data-parallel over batch
across 8 NeuronCores (2 images per core).

fp16 + software-pipelined design:
  - host converts inputs to fp16 (halves HBM traffic, doubles DVE rate),
    pre-scales M by 0.5, pads each image row with circular wrap columns,
    and stores row r as original row (r-1) mod H so block loads are single
    aligned [128,:] DMAs
  - 8 main row-blocks of 128 rows (no overlap); each yields out rows
    128t..128t+125.  The 2 missing rows per block (128t+126, 128t+127) are
    computed by one half-width "skinny" block: host packs 8 groups x 4
    consecutive rows per image into [64, 1028] tiles; 4x4 block-diagonal
    stencils produce the 2 interior rows of each group.
  - the loop is skewed into 3 stages so every engine always has ready work:
      S0(t): DMA loads (SP queue)
      S1(t): DVE aw/gw/vwu; PE psA=A@m, psD=F@mu; ACT dh=copy(psD);
             DVE vhu = psA*dh
      S2(t): PE psC = L@vhu + I@vwuR + (-I)@vwuL  (H-div + W-div folded);
             ACT out = copy(psC); GPSIMD store (SWDGE queue)
"""
import sys

sys.path.insert(0, "/opt/trn_rl_repo")

import numpy as np

B, H, W = 16, 1024, 1024
N_CORES = 8
IMGS_PER_CORE = B // N_CORES          # 2
PW = W + 2                            # padded row width (wrap cols)
NMAIN = 8                             # main blocks of 128 rows
NBLK = NMAIN + 1                      # + skinny block
HALF = 512                            # PSUM bank width (f32)
SKW = W + 4                           # skinny row width (2 extra wrap cols)

_CACHE = {}


def _build():
    import concourse.mybir as mybir
    from concourse import bacc
    from concourse.tile import TileContext

    f16 = mybir.dt.float16
    f32 = mybir.dt.float32
    Alu = mybir.AluOpType

    nc = bacc.Bacc(trn_type="TRN2", target_bir_lowering=False)

    M_d = nc.dram_tensor("m_in", [H, 2 * PW], f16, kind="ExternalInput")
    MU_d = nc.dram_tensor("mu_in", [H, 2 * PW], f16, kind="ExternalInput")
    MS_d = nc.dram_tensor("msk_in", [64, SKW], f16, kind="ExternalInput")
    MUS_d = nc.dram_tensor("musk_in", [64, SKW], f16, kind="ExternalInput")
    ST_d = nc.dram_tensor("stencils", [128, 8 * 128], f16, kind="ExternalInput")
    OUT_d = nc.dram_tensor("out", [H, 2 * W], f16, kind="ExternalOutput")
    OUTS_d = nc.dram_tensor("out_sk", [64, W], f16, kind="ExternalOutput")

    with TileContext(nc) as tc:
        with (
            tc.tile_pool(name="sb", bufs=2) as sbpool,
            tc.tile_pool(name="psA", bufs=1, space="PSUM") as poolA,
            tc.tile_pool(name="psDC", bufs=2, space="PSUM") as poolDC,
        ):
            st = sbpool.tile([128, 8 * 128], f16, tag="st", name="st", bufs=1)
            nc.scalar.dma_start(out=st[:], in_=ST_d[:])
            lA = st[:, 0:128]      # (I+U).T      H forward-average (x2)
            lF = st[:, 128:256]    # (U-I).T      H forward-diff
            lL = st[:, 256:384]    # (I-D).T      H backward-diff (divergence)
            lI = st[:, 384:512]    # I            W-part fold (+vwu_right)
            lIn = st[:, 512:640]   # -I           W-part fold (-vwu_left)
            lA4 = st[0:64, 640:704]    # 4x4-block versions for skinny
            lF4 = st[0:64, 768:832]
            lL4 = st[0:64, 896:960]
            lI64 = st[0:64, 384:448]
            lIn64 = st[0:64, 512:576]

            tiles = {}

            # moving-slice starts: (pair, half) -> fp16 input col, out col
            def mcol(p, hh):
                img, colh = divmod(2 * p + hh, 2)
                return img * PW + 1 + colh * HALF, img * W + colh * HALF

            def S0(t):
                if t == NMAIN:
                    msk = sbpool.tile([64, SKW], f16, tag="msk", name="msk", bufs=1)
                    musk = sbpool.tile([64, SKW], f16, tag="musk", name="musk", bufs=1)
                    nc.sync.dma_start(out=musk[:], in_=MUS_d[:])
                    nc.sync.dma_start(out=msk[:], in_=MS_d[:])
                    tiles[t] = {"mu": musk, "m": msk}
                    return
                r0 = 128 * t
                mu_t = sbpool.tile([128, 2 * PW], f16, tag="mu", name="mu_t", bufs=4)
                m_t = sbpool.tile([128, 2 * PW], f16, tag="m", name="m_t", bufs=4)
                if t == 0:
                    # parallelize the cold first loads across both HWDGE rings
                    nc.sync.dma_start(out=mu_t[0:64, :], in_=MU_d[r0:r0 + 64, :])
                    nc.scalar.dma_start(
                        out=mu_t[64:128, :], in_=MU_d[r0 + 64:r0 + 128, :])
                    nc.sync.dma_start(out=m_t[0:64, :], in_=M_d[r0:r0 + 64, :])
                    nc.scalar.dma_start(
                        out=m_t[64:128, :], in_=M_d[r0 + 64:r0 + 128, :])
                else:
                    nc.sync.dma_start(out=mu_t[:], in_=MU_d[r0:r0 + 128, :])
                    nc.sync.dma_start(out=m_t[:], in_=M_d[r0:r0 + 128, :])
                tiles[t] = {"mu": mu_t, "m": m_t}

            def S1(t):
                if t == NMAIN:
                    S1sk(t)
                    return
                d = tiles[t]
                mu_t, m_t = d["mu"], d["m"]
                m3 = m_t[:].rearrange("p (j k) -> p j k", j=2)
                mu3 = mu_t[:].rearrange("p (j k) -> p j k", j=2)
                aw = sbpool.tile([128, 2 * (W + 1)], f16, tag="aw", name="aw")
                aw3 = aw[:].rearrange("p (j k) -> p j k", j=2)
                nc.vector.tensor_tensor(
                    out=aw3, in0=m3[:, :, 0:W + 1], in1=m3[:, :, 1:W + 2],
                    op=Alu.add)
                gw = sbpool.tile([128, 2 * (W + 1)], f16, tag="gw", name="gw")
                gw3 = gw[:].rearrange("p (j k) -> p j k", j=2)
                nc.vector.tensor_tensor(
                    out=gw3, in0=mu3[:, :, 1:W + 2], in1=mu3[:, :, 0:W + 1],
                    op=Alu.subtract)
                vwu = sbpool.tile([128, 2 * (W + 1)], f16, tag="vwu", name="vwu", bufs=4)
                nc.vector.tensor_tensor(
                    out=vwu[:], in0=aw[:], in1=gw[:], op=Alu.mult)

                # F(pair0) + dh0 first so vhu's last input lands early;
                # A-matmuls run while ACT copies dh0 (psD bufs=1 reuse)
                dh = sbpool.tile([128, 2 * W], f16, tag="dh", name="dh")
                psA = poolA.tile([128, 4 * HALF], f32, tag="psA", name="psA")
                psD0 = poolDC.tile([128, 2 * HALF], f32, tag="psDC", name="psD0")
                for hh in range(2):
                    c0, _ = mcol(0, hh)
                    nc.tensor.matmul(
                        psD0[:, hh * HALF:(hh + 1) * HALF], lF,
                        mu_t[:, c0:c0 + HALF], start=True, stop=True)
                nc.scalar.copy(out=dh[:, 0:1024], in_=psD0[:])
                for p in range(2):
                    for hh in range(2):
                        c0, o0 = mcol(p, hh)
                        nc.tensor.matmul(
                            psA[:, o0:o0 + HALF], lA,
                            m_t[:, c0:c0 + HALF], start=True, stop=True)
                psD1 = poolDC.tile([128, 2 * HALF], f32, tag="psDC", name="psD1")
                for hh in range(2):
                    c0, _ = mcol(1, hh)
                    nc.tensor.matmul(
                        psD1[:, hh * HALF:(hh + 1) * HALF], lF,
                        mu_t[:, c0:c0 + HALF], start=True, stop=True)
                nc.scalar.copy(out=dh[:, 1024:2048], in_=psD1[:])
                vhu = sbpool.tile([128, 2 * W], f16, tag="vhu", name="vhu", bufs=4)
                # low half: PSUM read at DVE 1x; high half: ACT copies ah to
                # fp16 so the product runs at DVE 2x (DVE/ACT balance)
                nc.vector.tensor_tensor(
                    out=vhu[:, 0:1024], in0=psA[:, 0:1024],
                    in1=dh[:, 0:1024], op=Alu.mult)
                ah = sbpool.tile([128, 1024], f16, tag="ah", name="ah")
                nc.scalar.copy(out=ah[:], in_=psA[:, 1024:2048])
                nc.vector.tensor_tensor(
                    out=vhu[:, 1024:2048], in0=ah[:],
                    in1=dh[:, 1024:2048], op=Alu.mult)
                d["vwu"] = vwu
                d["vhu"] = vhu

            def S1sk(t):
                d = tiles[t]
                musk, msk = d["mu"], d["m"]
                aw = sbpool.tile([64, W + 2], f16, tag="awk", name="awk")
                nc.vector.tensor_tensor(
                    out=aw[:], in0=msk[:, 0:W + 2], in1=msk[:, 1:W + 3],
                    op=Alu.add)
                gw = sbpool.tile([64, W + 2], f16, tag="gwk", name="gwk")
                nc.vector.tensor_tensor(
                    out=gw[:], in0=musk[:, 1:W + 3], in1=musk[:, 0:W + 2],
                    op=Alu.subtract)
                vwu = sbpool.tile([64, W + 2], f16, tag="vwk", name="vwk", bufs=2)
                nc.vector.tensor_tensor(
                    out=vwu[:], in0=aw[:], in1=gw[:], op=Alu.mult)

                psA = poolA.tile([64, 2 * HALF], f32, tag="psA", name="psAk")
                for hh in range(2):
                    nc.tensor.matmul(
                        psA[:, hh * HALF:(hh + 1) * HALF], lA4,
                        msk[:, 1 + hh * HALF:1 + (hh + 1) * HALF],
                        start=True, stop=True)
                psD = poolDC.tile([64, 2 * HALF], f32, tag="psDC", name="psDk")
                for hh in range(2):
                    nc.tensor.matmul(
                        psD[:, hh * HALF:(hh + 1) * HALF], lF4,
                        musk[:, 1 + hh * HALF:1 + (hh + 1) * HALF],
                        start=True, stop=True)
                dh = sbpool.tile([64, W], f16, tag="dhk", name="dhk")
                nc.scalar.copy(out=dh[:], in_=psD[:])
                vhu = sbpool.tile([64, W], f16, tag="vhuk", name="vhuk", bufs=2)
                nc.vector.tensor_tensor(
                    out=vhu[:], in0=psA[:], in1=dh[:], op=Alu.mult)
                d["vwu"] = vwu
                d["vhu"] = vhu

            def S2(t):
                if t == NMAIN:
                    S2sk(t)
                    return
                d = tiles[t]
                vwu, vhu = d["vwu"], d["vhu"]
                out_t = sbpool.tile([128, 2 * W], f16, tag="out", name="out_t", bufs=4)
                for p in range(2):
                    psC = poolDC.tile([128, 2 * HALF], f32, tag="psDC", name="psC")
                    for hh in range(2):
                        _, o0 = mcol(p, hh)
                        nc.tensor.matmul(
                            psC[:, hh * HALF:(hh + 1) * HALF], lL,
                            vhu[:, o0:o0 + HALF], start=True, stop=False)
                    for hh in range(2):
                        img, colh = divmod(2 * p + hh, 2)
                        cR = img * (W + 1) + colh * HALF + 1
                        nc.tensor.matmul(
                            psC[:, hh * HALF:(hh + 1) * HALF], lI,
                            vwu[:, cR:cR + HALF], start=False, stop=False)
                    for hh in range(2):
                        img, colh = divmod(2 * p + hh, 2)
                        cL = img * (W + 1) + colh * HALF
                        nc.tensor.matmul(
                            psC[:, hh * HALF:(hh + 1) * HALF], lIn,
                            vwu[:, cL:cL + HALF], start=False, stop=True)
                    if p == 1 and t >= NMAIN - 2:
                        # pipeline tail: ACT is the serializer here and DVE
                        # is idle -- overlap the two out-copies
                        nc.vector.tensor_copy(
                            out=out_t[:, 1024:2048], in_=psC[:])
                    else:
                        nc.scalar.copy(
                            out=out_t[:, p * 1024:(p + 1) * 1024], in_=psC[:])
                    # store this image's rows as soon as its copy lands;
                    # alternate queues so final completions drain in parallel
                    r_out = 128 * t
                    eng = nc.gpsimd if p == 0 else nc.sync
                    eng.dma_start(
                        out=OUT_d[r_out:r_out + 126, p * W:(p + 1) * W],
                        in_=out_t[1:127, p * 1024:(p + 1) * 1024])
                del tiles[t]

            def S2sk(t):
                d = tiles[t]
                vwu, vhu = d["vwu"], d["vhu"]
                out_t = sbpool.tile([64, W], f16, tag="outk", name="outk", bufs=2)
                psC = poolDC.tile([64, 2 * HALF], f32, tag="psDC", name="psCk")
                for hh in range(2):
                    nc.tensor.matmul(
                        psC[:, hh * HALF:(hh + 1) * HALF], lL4,
                        vhu[:, hh * HALF:(hh + 1) * HALF],
                        start=True, stop=False)
                for hh in range(2):
                    nc.tensor.matmul(
                        psC[:, hh * HALF:(hh + 1) * HALF], lI64,
                        vwu[:, 1 + hh * HALF:1 + (hh + 1) * HALF],
                        start=False, stop=False)
                for hh in range(2):
                    nc.tensor.matmul(
                        psC[:, hh * HALF:(hh + 1) * HALF], lIn64,
                        vwu[:, hh * HALF:hh * HALF + HALF],
                        start=False, stop=True)
                nc.scalar.copy(out=out_t[:], in_=psC[:])
                nc.gpsimd.dma_start(out=OUTS_d[:], in_=out_t[:])
                del tiles[t]

            # skinny block first: its small loads land fast, so its compute
            # fills the pipeline-fill phase; the tail is then one main S2
            order = [NMAIN] + list(range(NMAIN))
            for r in range(NBLK + 2):
                if r < NBLK:
                    S0(order[r])
                if 1 <= r <= NBLK:
                    S1(order[r - 1])
                if r >= 2:
                    S2(order[r - 2])

    nc.compile()
    return nc


def _stencils():
    A = np.zeros((128, 128), dtype=np.float32)
    F = np.zeros((128, 128), dtype=np.float32)
    L = np.zeros((128, 128), dtype=np.float32)
    for r in range(127):
        A[r, r] = 1.0
        A[r, r + 1] = 1.0
        F[r, r] = -1.0
        F[r, r + 1] = 1.0
    A[127, 127] = 1.0
    F[127, 127] = -1.0
    for r in range(128):
        L[r, r] = 1.0
    for r in range(1, 128):
        L[r, r - 1] = -1.0
    # skinny 4x4-block versions (16 groups in 64x64, embedded top-left)
    A4 = np.zeros((128, 128), dtype=np.float32)
    F4 = np.zeros((128, 128), dtype=np.float32)
    L4 = np.zeros((128, 128), dtype=np.float32)
    for g in range(16):
        b = 4 * g
        for p in range(3):
            A4[b + p, b + p] = 1.0
            A4[b + p, b + p + 1] = 1.0
            F4[b + p, b + p] = -1.0
            F4[b + p, b + p + 1] = 1.0
        A4[b + 3, b + 3] = 1.0
        F4[b + 3, b + 3] = -1.0
        for p in (1, 2):
            L4[b + p, b + p] = 1.0
            L4[b + p, b + p - 1] = -1.0
    st = np.zeros((128, 8 * 128), dtype=np.float32)
    st[:, 0:128] = A.T
    st[:, 128:256] = F.T
    st[:, 256:384] = L.T
    st[:, 384:512] = np.eye(128, dtype=np.float32)
    st[:, 512:640] = -np.eye(128, dtype=np.float32)
    st[:, 640:768] = A4.T
    st[:, 768:896] = F4.T
    st[:, 896:1024] = L4.T
    return st.astype(np.float16)


def _pad_rows(x):
    """[2, H, W] fp16 -> [H, 2*(W+2)]: circular wrap columns + row r holds
    original row (r-1) mod H so block loads are single aligned DMAs."""
    out = np.empty((H, 2, PW), dtype=np.float16)
    for j in range(2):
        out[:, j, 1:W + 1] = x[j]
        out[:, j, 0] = x[j][:, W - 1]
        out[:, j, W + 1] = x[j][:, 0]
    flat = out.reshape(H, 2 * PW)
    idx = (np.arange(H) - 1) % H
    return np.ascontiguousarray(flat[idx])


def _pack_skinny(x):
    """[2, H, W] fp16 -> [64, W+4]: image j in partitions 32j..32j+31;
    group g = rows 128g+125..128g+128 (mod H); wrap cols on both sides."""
    out = np.zeros((64, SKW), dtype=np.float16)
    for j in range(2):
        for g in range(8):
            for p in range(4):
                row = (128 * g + 125 + p) % H
                dst = out[32 * j + 4 * g + p]
                dst[1:W + 1] = x[j][row]
                dst[0] = x[j][row][W - 1]
                dst[W + 1] = x[j][row][0]
                dst[W + 2] = x[j][row][1]
    return out


def make_in_maps(inputs):
    M = np.asarray(inputs["M"], dtype=np.float32).reshape(B, H, W)
    mu = np.asarray(inputs["mu"], dtype=np.float32).reshape(B, H, W)
    st = _stencils()
    in_maps = []
    for c in range(N_CORES):
        i0 = c * IMGS_PER_CORE
        ms = (M[i0:i0 + 2] * 0.5).astype(np.float16)
        mus = mu[i0:i0 + 2].astype(np.float16)
        in_maps.append({
            "m_in": _pad_rows(ms),
            "mu_in": _pad_rows(mus),
            "msk_in": _pack_skinny(ms),
            "musk_in": _pack_skinny(mus),
            "stencils": st,
        })
    return in_maps


def kernel(M, mu):
    from concourse.bass_utils import run_bass_kernel_spmd

    if "nc" not in _CACHE:
        _CACHE["nc"] = _build()
    nc = _CACHE["nc"]

    in_maps = make_in_maps({"M": M, "mu": mu})

    res = run_bass_kernel_spmd(nc, in_maps, core_ids=list(range(N_CORES)))
    out = np.empty((B, H, W), dtype=np.float32)
    for c in range(N_CORES):
        o = res.results[c]["out"].reshape(H, 2, W)
        osk = res.results[c]["out_sk"]
        for j in range(IMGS_PER_CORE):
            img = o[:, j, :].astype(np.float32)
            for g in range(8):
                img[128 * g + 126] = osk[32 * j + 4 * g + 1].astype(np.float32)
                img[128 * g + 127] = osk[32 * j + 4 * g + 2].astype(np.float32)
            out[c * IMGS_PER_CORE + j] = img
    return out.reshape(B, 1, H, W)


# revision 38
# speedup vs baseline: 1.0071x; 1.0071x over previous
"""Trainium2 Bass kernel for the periodic flux-divergence stencil:

    out = sum_ax  (v - roll(v, 1, ax)),  v = 0.5*(roll(M,-1,ax)+M)*(roll(mu,-1,ax)-mu)

over axes H, W of [B=16, 1, 1024, 1024] f32 inputs, data-parallel over batch
across 8 NeuronCores (2 images per core).

fp16 + software-pipelined design:
  - host converts inputs to fp16 (halves HBM traffic, doubles DVE rate),
    pre-scales M by 0.5, pads each image row with circular wrap columns,
    and stores row r as original row (r-1) mod H so block loads are single
    aligned [128,:] DMAs
  - 8 main row-blocks of 128 rows (no overlap); each yields out rows
    128t..128t+125.  The 2 missing rows per block (128t+126, 128t+127) are
    computed by one half-width "skinny" block: host packs 8 groups x 4
    consecutive rows per image into [64, 1028] tiles; 4x4 block-diagonal
    stencils produce the 2 interior rows of each group.
  - the loop is skewed into 3 stages so every engine always has ready work:
      S0(t): DMA loads (SP queue)
      S1(t): DVE aw/gw/vwu; PE psA=A@m, psD=F@mu; ACT dh=copy(psD);
             DVE vhu = psA*dh
      S2(t): PE psC = L@vhu + I@vwuR + (-I)@vwuL  (H-div + W-div folded);
             ACT out = copy(psC); GPSIMD store (SWDGE queue)
"""
import sys

sys.path.insert(0, "/opt/trn_rl_repo")

import numpy as np

B, H, W = 16, 1024, 1024
N_CORES = 8
IMGS_PER_CORE = B // N_CORES          # 2
PW = W + 2                            # padded row width (wrap cols)
NMAIN = 8                             # main blocks of 128 rows
NBLK = NMAIN + 1                      # + skinny block
HALF = 512                            # PSUM bank width (f32)
SKW = W + 4                           # skinny row width (2 extra wrap cols)

_CACHE = {}


def _build():
    import concourse.mybir as mybir
    from concourse import bacc
    from concourse.tile import TileContext

    f16 = mybir.dt.float16
    f32 = mybir.dt.float32
    Alu = mybir.AluOpType

    nc = bacc.Bacc(trn_type="TRN2", target_bir_lowering=False)

    M_d = nc.dram_tensor("m_in", [H, 2 * PW], f16, kind="ExternalInput")
    MU_d = nc.dram_tensor("mu_in", [H, 2 * PW], f16, kind="ExternalInput")
    MS_d = nc.dram_tensor("msk_in", [64, SKW], f16, kind="ExternalInput")
    MUS_d = nc.dram_tensor("musk_in", [64, SKW], f16, kind="ExternalInput")
    ST_d = nc.dram_tensor("stencils", [128, 8 * 128], f16, kind="ExternalInput")
    OUT_d = nc.dram_tensor("out", [H, 2 * W], f16, kind="ExternalOutput")
    OUTS_d = nc.dram_tensor("out_sk", [64, W], f16, kind="ExternalOutput")

    with TileContext(nc) as tc:
        with (
            tc.tile_pool(name="sb", bufs=2) as sbpool,
            tc.tile_pool(name="psA", bufs=1, space="PSUM") as poolA,
            tc.tile_pool(name="psDC", bufs=2, space="PSUM") as poolDC,
        ):
            st = sbpool.tile([128, 8 * 128], f16, tag="st", name="st", bufs=1)
            nc.scalar.dma_start(out=st[:], in_=ST_d[:])
            lA = st[:, 0:128]      # (I+U).T      H forward-average (x2)
            lF = st[:, 128:256]    # (U-I).T      H forward-diff
            lL = st[:, 256:384]    # (I-D).T      H backward-diff (divergence)
            lI = st[:, 384:512]    # I            W-part fold (+vwu_right)
            lIn = st[:, 512:640]   # -I           W-part fold (-vwu_left)
            lA4 = st[0:64, 640:704]    # 4x4-block versions for skinny
            lF4 = st[0:64, 768:832]
            lL4 = st[0:64, 896:960]
            lI64 = st[0:64, 384:448]
            lIn64 = st[0:64, 512:576]

            tiles = {}

            # moving-slice starts: (pair, half) -> fp16 input col, out col
            def mcol(p, hh):
                img, colh = divmod(2 * p + hh, 2)
                return img * PW + 1 + colh * HALF, img * W + colh * HALF

            def S0(t):
                if t == NMAIN:
                    msk = sbpool.tile([64, SKW], f16, tag="msk", name="msk", bufs=1)
                    musk = sbpool.tile([64, SKW], f16, tag="musk", name="musk", bufs=1)
                    nc.sync.dma_start(out=musk[:], in_=MUS_d[:])
                    nc.sync.dma_start(out=msk[:], in_=MS_d[:])
                    tiles[t] = {"mu": musk, "m": msk}
                    return
                r0 = 128 * t
                mu_t = sbpool.tile([128, 2 * PW], f16, tag="mu", name="mu_t", bufs=4)
                m_t = sbpool.tile([128, 2 * PW], f16, tag="m", name="m_t", bufs=4)
                if t == 0:
                    # parallelize the cold first loads across both HWDGE rings
                    nc.sync.dma_start(out=mu_t[0:64, :], in_=MU_d[r0:r0 + 64, :])
                    nc.scalar.dma_start(
                        out=mu_t[64:128, :], in_=MU_d[r0 + 64:r0 + 128, :])
                    nc.sync.dma_start(out=m_t[0:64, :], in_=M_d[r0:r0 + 64, :])
                    nc.scalar.dma_start(
                        out=m_t[64:128, :], in_=M_d[r0 + 64:r0 + 128, :])
                elif t <= 2:
                    # fill phase: ACT is idle, use its HWDGE ring for m
                    nc.sync.dma_start(out=mu_t[:], in_=MU_d[r0:r0 + 128, :])
                    nc.scalar.dma_start(out=m_t[:], in_=M_d[r0:r0 + 128, :])
                else:
                    nc.sync.dma_start(out=mu_t[:], in_=MU_d[r0:r0 + 128, :])
                    nc.sync.dma_start(out=m_t[:], in_=M_d[r0:r0 + 128, :])
                tiles[t] = {"mu": mu_t, "m": m_t}

            def S1(t):
                if t == NMAIN:
                    S1sk(t)
                    return
                d = tiles[t]
                mu_t, m_t = d["mu"], d["m"]
                m3 = m_t[:].rearrange("p (j k) -> p j k", j=2)
                mu3 = mu_t[:].rearrange("p (j k) -> p j k", j=2)
                aw = sbpool.tile([128, 2 * (W + 1)], f16, tag="aw", name="aw")
                aw3 = aw[:].rearrange("p (j k) -> p j k", j=2)
                nc.vector.tensor_tensor(
                    out=aw3, in0=m3[:, :, 0:W + 1], in1=m3[:, :, 1:W + 2],
                    op=Alu.add)
                gw = sbpool.tile([128, 2 * (W + 1)], f16, tag="gw", name="gw")
                gw3 = gw[:].rearrange("p (j k) -> p j k", j=2)
                nc.vector.tensor_tensor(
                    out=gw3, in0=mu3[:, :, 1:W + 2], in1=mu3[:, :, 0:W + 1],
                    op=Alu.subtract)
                vwu = sbpool.tile([128, 2 * (W + 1)], f16, tag="vwu", name="vwu", bufs=4)
                nc.vector.tensor_tensor(
                    out=vwu[:], in0=aw[:], in1=gw[:], op=Alu.mult)

                # F(pair0) + dh0 first so vhu's last input lands early;
                # A-matmuls run while ACT copies dh0 (psD bufs=1 reuse)
                dh = sbpool.tile([128, 2 * W], f16, tag="dh", name="dh")
                psA = poolA.tile([128, 4 * HALF], f32, tag="psA", name="psA")
                psD0 = poolDC.tile([128, 2 * HALF], f32, tag="psDC", name="psD0")
                for hh in range(2):
                    c0, _ = mcol(0, hh)
                    nc.tensor.matmul(
                        psD0[:, hh * HALF:(hh + 1) * HALF], lF,
                        mu_t[:, c0:c0 + HALF], start=True, stop=True)
                nc.scalar.copy(out=dh[:, 0:1024], in_=psD0[:])
                for p in range(2):
                    for hh in range(2):
                        c0, o0 = mcol(p, hh)
                        nc.tensor.matmul(
                            psA[:, o0:o0 + HALF], lA,
                            m_t[:, c0:c0 + HALF], start=True, stop=True)
                psD1 = poolDC.tile([128, 2 * HALF], f32, tag="psDC", name="psD1")
                for hh in range(2):
                    c0, _ = mcol(1, hh)
                    nc.tensor.matmul(
                        psD1[:, hh * HALF:(hh + 1) * HALF], lF,
                        mu_t[:, c0:c0 + HALF], start=True, stop=True)
                nc.scalar.copy(out=dh[:, 1024:2048], in_=psD1[:])
                vhu = sbpool.tile([128, 2 * W], f16, tag="vhu", name="vhu", bufs=4)
                # low half: PSUM read at DVE 1x; high half: ACT copies ah to
                # fp16 so the product runs at DVE 2x (DVE/ACT balance)
                nc.vector.tensor_tensor(
                    out=vhu[:, 0:1280], in0=psA[:, 0:1280],
                    in1=dh[:, 0:1280], op=Alu.mult)
                ah = sbpool.tile([128, 768], f16, tag="ah", name="ah")
                nc.scalar.copy(out=ah[:], in_=psA[:, 1280:2048])
                nc.vector.tensor_tensor(
                    out=vhu[:, 1280:2048], in0=ah[:],
                    in1=dh[:, 1280:2048], op=Alu.mult)
                d["vwu"] = vwu
                d["vhu"] = vhu

            def S1sk(t):
                d = tiles[t]
                musk, msk = d["mu"], d["m"]
                aw = sbpool.tile([64, W + 2], f16, tag="awk", name="awk")
                nc.vector.tensor_tensor(
                    out=aw[:], in0=msk[:, 0:W + 2], in1=msk[:, 1:W + 3],
                    op=Alu.add)
                gw = sbpool.tile([64, W + 2], f16, tag="gwk", name="gwk")
                nc.vector.tensor_tensor(
                    out=gw[:], in0=musk[:, 1:W + 3], in1=musk[:, 0:W + 2],
                    op=Alu.subtract)
                vwu = sbpool.tile([64, W + 2], f16, tag="vwk", name="vwk", bufs=2)
                nc.vector.tensor_tensor(
                    out=vwu[:], in0=aw[:], in1=gw[:], op=Alu.mult)

                psA = poolA.tile([64, 2 * HALF], f32, tag="psA", name="psAk")
                for hh in range(2):
                    nc.tensor.matmul(
                        psA[:, hh * HALF:(hh + 1) * HALF], lA4,
                        msk[:, 1 + hh * HALF:1 + (hh + 1) * HALF],
                        start=True, stop=True)
                psD = poolDC.tile([64, 2 * HALF], f32, tag="psDC", name="psDk")
                for hh in range(2):
                    nc.tensor.matmul(
                        psD[:, hh * HALF:(hh + 1) * HALF], lF4,
                        musk[:, 1 + hh * HALF:1 + (hh + 1) * HALF],
                        start=True, stop=True)
                dh = sbpool.tile([64, W], f16, tag="dhk", name="dhk")
                nc.scalar.copy(out=dh[:], in_=psD[:])
                vhu = sbpool.tile([64, W], f16, tag="vhuk", name="vhuk", bufs=2)
                nc.vector.tensor_tensor(
                    out=vhu[:], in0=psA[:], in1=dh[:], op=Alu.mult)
                d["vwu"] = vwu
                d["vhu"] = vhu

            def S2(t):
                if t == NMAIN:
                    S2sk(t)
                    return
                d = tiles[t]
                vwu, vhu = d["vwu"], d["vhu"]
                out_t = sbpool.tile([128, 2 * W], f16, tag="out", name="out_t", bufs=4)
                for p in range(2):
                    psC = poolDC.tile([128, 2 * HALF], f32, tag="psDC", name="psC")
                    for hh in range(2):
                        _, o0 = mcol(p, hh)
                        nc.tensor.matmul(
                            psC[:, hh * HALF:(hh + 1) * HALF], lL,
                            vhu[:, o0:o0 + HALF], start=True, stop=False)
                    for hh in range(2):
                        img, colh = divmod(2 * p + hh, 2)
                        cR = img * (W + 1) + colh * HALF + 1
                        nc.tensor.matmul(
                            psC[:, hh * HALF:(hh + 1) * HALF], lI,
                            vwu[:, cR:cR + HALF], start=False, stop=False)
                    for hh in range(2):
                        img, colh = divmod(2 * p + hh, 2)
                        cL = img * (W + 1) + colh * HALF
                        nc.tensor.matmul(
                            psC[:, hh * HALF:(hh + 1) * HALF], lIn,
                            vwu[:, cL:cL + HALF], start=False, stop=True)
                    if p == 1 and t >= NMAIN - 2:
                        # pipeline tail: ACT is the serializer here and DVE
                        # is idle -- overlap the two out-copies
                        nc.vector.tensor_copy(
                            out=out_t[:, 1024:2048], in_=psC[:])
                    else:
                        nc.scalar.copy(
                            out=out_t[:, p * 1024:(p + 1) * 1024], in_=psC[:])
                    # store this image's rows as soon as its copy lands;
                    # alternate queues so final completions drain in parallel
                    r_out = 128 * t
                    eng = nc.gpsimd if p == 0 else nc.sync
                    eng.dma_start(
                        out=OUT_d[r_out:r_out + 126, p * W:(p + 1) * W],
                        in_=out_t[1:127, p * 1024:(p + 1) * 1024])
                del tiles[t]

            def S2sk(t):
                d = tiles[t]
                vwu, vhu = d["vwu"], d["vhu"]
                out_t = sbpool.tile([64, W], f16, tag="outk", name="outk", bufs=2)
                psC = poolDC.tile([64, 2 * HALF], f32, tag="psDC", name="psCk")
                for hh in range(2):
                    nc.tensor.matmul(
                        psC[:, hh * HALF:(hh + 1) * HALF], lL4,
                        vhu[:, hh * HALF:(hh + 1) * HALF],
                        start=True, stop=False)
                for hh in range(2):
                    nc.tensor.matmul(
                        psC[:, hh * HALF:(hh + 1) * HALF], lI64,
                        vwu[:, 1 + hh * HALF:1 + (hh + 1) * HALF],
                        start=False, stop=False)
                for hh in range(2):
                    nc.tensor.matmul(
                        psC[:, hh * HALF:(hh + 1) * HALF], lIn64,
                        vwu[:, hh * HALF:hh * HALF + HALF],
                        start=False, stop=True)
                nc.scalar.copy(out=out_t[:], in_=psC[:])
                nc.gpsimd.dma_start(out=OUTS_d[:], in_=out_t[:])
                del tiles[t]

            # skinny block first: its small loads land fast, so its compute
            # fills the pipeline-fill phase; the tail is then one main S2
            order = [NMAIN] + list(range(NMAIN))
            for r in range(NBLK + 2):
                if r < NBLK:
                    S0(order[r])
                if 1 <= r <= NBLK:
                    S1(order[r - 1])
                if r >= 2:
                    S2(order[r - 2])

    nc.compile()
    return nc


def _stencils():
    A = np.zeros((128, 128), dtype=np.float32)
    F = np.zeros((128, 128), dtype=np.float32)
    L = np.zeros((128, 128), dtype=np.float32)
    for r in range(127):
        A[r, r] = 1.0
        A[r, r + 1] = 1.0
        F[r, r] = -1.0
        F[r, r + 1] = 1.0
    A[127, 127] = 1.0
    F[127, 127] = -1.0
    for r in range(128):
        L[r, r] = 1.0
    for r in range(1, 128):
        L[r, r - 1] = -1.0
    # skinny 4x4-block versions (16 groups in 64x64, embedded top-left)
    A4 = np.zeros((128, 128), dtype=np.float32)
    F4 = np.zeros((128, 128), dtype=np.float32)
    L4 = np.zeros((128, 128), dtype=np.float32)
    for g in range(16):
        b = 4 * g
        for p in range(3):
            A4[b + p, b + p] = 1.0
            A4[b + p, b + p + 1] = 1.0
            F4[b + p, b + p] = -1.0
            F4[b + p, b + p + 1] = 1.0
        A4[b + 3, b + 3] = 1.0
        F4[b + 3, b + 3] = -1.0
        for p in (1, 2):
            L4[b + p, b + p] = 1.0
            L4[b + p, b + p - 1] = -1.0
    st = np.zeros((128, 8 * 128), dtype=np.float32)
    st[:, 0:128] = A.T
    st[:, 128:256] = F.T
    st[:, 256:384] = L.T
    st[:, 384:512] = np.eye(128, dtype=np.float32)
    st[:, 512:640] = -np.eye(128, dtype=np.float32)
    st[:, 640:768] = A4.T
    st[:, 768:896] = F4.T
    st[:, 896:1024] = L4.T
    return st.astype(np.float16)


def _pad_rows(x):
    """[2, H, W] fp16 -> [H, 2*(W+2)]: circular wrap columns + row r holds
    original row (r-1) mod H so block loads are single aligned DMAs."""
    out = np.empty((H, 2, PW), dtype=np.float16)
    for j in range(2):
        out[:, j, 1:W + 1] = x[j]
        out[:, j, 0] = x[j][:, W - 1]
        out[:, j, W + 1] = x[j][:, 0]
    flat = out.reshape(H, 2 * PW)
    idx = (np.arange(H) - 1) % H
    return np.ascontiguousarray(flat[idx])


def _pack_skinny(x):
    """[2, H, W] fp16 -> [64, W+4]: image j in partitions 32j..32j+31;
    group g = rows 128g+125..128g+128 (mod H); wrap cols on both sides."""
    out = np.zeros((64, SKW), dtype=np.float16)
    for j in range(2):
        for g in range(8):
            for p in range(4):
                row = (128 * g + 125 + p) % H
                dst = out[32 * j + 4 * g + p]
                dst[1:W + 1] = x[j][row]
                dst[0] = x[j][row][W - 1]
                dst[W + 1] = x[j][row][0]
                dst[W + 2] = x[j][row][1]
    return out


def make_in_maps(inputs):
    M = np.asarray(inputs["M"], dtype=np.float32).reshape(B, H, W)
    mu = np.asarray(inputs["mu"], dtype=np.float32).reshape(B, H, W)
    st = _stencils()
    in_maps = []
    for c in range(N_CORES):
        i0 = c * IMGS_PER_CORE
        ms = (M[i0:i0 + 2] * 0.5).astype(np.float16)
        mus = mu[i0:i0 + 2].astype(np.float16)
        in_maps.append({
            "m_in": _pad_rows(ms),
            "mu_in": _pad_rows(mus),
            "msk_in": _pack_skinny(ms),
            "musk_in": _pack_skinny(mus),
            "stencils": st,
        })
    return in_maps


def kernel(M, mu):
    from concourse.bass_utils import run_bass_kernel_spmd

    if "nc" not in _CACHE:
        _CACHE["nc"] = _build()
    nc = _CACHE["nc"]

    in_maps = make_in_maps({"M": M, "mu": mu})

    res = run_bass_kernel_spmd(nc, in_maps, core_ids=list(range(N_CORES)))
    out = np.empty((B, H, W), dtype=np.float32)
    for c in range(N_CORES):
        o = res.results[c]["out"].reshape(H, 2, W)
        osk = res.results[c]["out_sk"]
        for j in range(IMGS_PER_CORE):
            img = o[:, j, :].astype(np.float32)
            for g in range(8):
                img[128 * g + 126] = osk[32 * j + 4 * g + 1].astype(np.float32)
                img[128 * g + 127] = osk[32 * j + 4 * g + 2].astype(np.float32)
            out[c * IMGS_PER_CORE + j] = img
    return out.reshape(B, 1, H, W)
